# revision 1
# baseline (speedup 1.0000x reference)
"""GuidedAttentionLoss on 8 Trainium2 NeuronCores (Bass/Tile).

loss = mean(guide * a^T) over [B=64, T=2048, N=512], where
  guide[b,t,k] = (1 - exp(-((k - floor(N_b/T_b * t))/N_b)^2 / (2*sigma^2)))
                 for t < T_b, k < N_b; 0 elsewhere.

Strategy (pure data parallel, 8 batches per core):
  * Host knows the integer lengths, so the per-(b,t) guide coefficients are
    precomputed on host as tiny tensors:
      scaled squared distance  c*(n - o_t)^2 = n^2*R0[t] + n*R1[t] + R2[t]
    with R0=c, R1=-2c*o_t, R2=c*o_t^2 and L=[n^2, n, 1]; the PE computes each
    [128, w] "sq" tile as a K=3 fp32 matmul (L^T @ R) into PSUM.
  * ScalarE: e = Exp(-sq)  (single activation pass, PSUM->SBUF).
  * VectorE: one scalar_tensor_tensor per tile: out=(e-1)*a with accum_out
    giving per-partition sums of (e-1)*a == -a*guide.  That single op fuses
    the (1-e) fold, the product with a, and the reduction.
  * Host: loss = -sum(accums over cores) / (B*N*T) in f64.
  * Only the valid [N_b, T_b] rectangle is streamed; outside it the host
    zero-pads the staged input so padded/junk lanes contribute exactly 0.
  * One SPMD program for all 8 cores: the 64 batches are dealt into 8
    "slots" x 8 cores (swap hill-climb minimizing sum(maxN*maxT)) so every
    instruction's shape is the per-slot max — small padding, perfectly
    balanced cores.
"""

import numpy as np

B, N_MAX, T_MAX = 64, 512, 2048
SIGMA = 0.4
N_CORES = 8
PART = 128
CHUNK = 512  # max fp32 matmul moving free dim == one PSUM bank


def _plan(input_lengths: np.ndarray, target_lengths: np.ndarray):
    """Assign the B batches to (slot, core) so all cores share shapes.

    Returns list of (idxs[n_cores], tiles, T_slot) per slot, where tiles is
    the list of row-tile heights ([128, ..., partial]) covering max(N) of the
    slot and T_slot = max(T).  Assignment minimizes sum(maxN*maxT) (the
    per-core padded DMA volume) with a swap hill-climb from a sorted start.
    """
    Ns = input_lengths.astype(np.int64)
    Ts = target_lengths.astype(np.int64)
    assert Ns.shape == (B,) and Ts.shape == (B,)
    n_slots = B // N_CORES

    def slot_cost(g):
        return int(max(Ns[i] for i in g)) * int(max(Ts[i] for i in g))

    def sweep(groups):
        # full pairwise-swap local search to convergence
        improved = True
        while improved:
            improved = False
            for s1 in range(n_slots):
                for s2 in range(s1 + 1, n_slots):
                    g1, g2 = groups[s1], groups[s2]
                    for i1 in range(N_CORES):
                        for i2 in range(N_CORES):
                            c0 = slot_cost(g1) + slot_cost(g2)
                            g1[i1], g2[i2] = g2[i2], g1[i1]
                            if slot_cost(g1) + slot_cost(g2) < c0:
                                improved = True
                            else:
                                g1[i1], g2[i2] = g2[i2], g1[i1]
        return groups

    rng = np.random.default_rng(0)
    best_cost, groups = None, None
    for trial in range(12):
        if trial == 0:
            order = np.argsort(-(Ns * Ts))
        elif trial == 1:
            order = np.argsort(-Ts)
        elif trial == 2:
            order = np.argsort(-Ns)
        else:
            order = rng.permutation(B)
        cand = sweep(
            [list(order[s * N_CORES : (s + 1) * N_CORES]) for s in range(n_slots)]
        )
        c = sum(slot_cost(g) for g in cand)
        if best_cost is None or c < best_cost:
            best_cost, groups = c, [list(g) for g in cand]

    slots = []
    for g in groups:
        idxs = np.array(g)
        maxN = int(Ns[idxs].max())
        T_slot = int(Ts[idxs].max())
        tiles = [PART] * (maxN // PART)
        if maxN % PART:
            tiles.append(maxN % PART)
        slots.append((idxs, tiles, T_slot))
    return slots


def _host_inputs(alignments, input_lengths, target_lengths, slots):
    """Per-core input dicts for run_bass_kernel_spmd."""
    tot_rows = sum(sum(tiles) for _, tiles, _ in slots)
    n_slots = len(slots)
    t = np.arange(T_MAX, dtype=np.float32)

    lmat = np.zeros((3, N_MAX), dtype=np.float32)
    j = np.arange(N_MAX, dtype=np.float32)
    lmat[0] = j * j
    lmat[1] = j
    lmat[2] = 1.0

    in_maps = []
    for core in range(N_CORES):
        A = np.zeros((tot_rows, T_MAX), dtype=np.float32)
        R = np.zeros((n_slots * 3, T_MAX), dtype=np.float32)
        row0 = 0
        for s, (idxs, tiles, T_slot) in enumerate(slots):
            b = int(idxs[core])
            Nb = int(input_lengths[b])
            Tb = int(target_lengths[b])
            A[row0 : row0 + Nb, :Tb] = alignments[b, :Nb, :Tb]
            # match reference f32 arithmetic: floor(float32(N)/float32(T) * t)
            ratio = np.float32(Nb) / np.float32(Tb)
            o = np.floor(ratio * t)
            c = np.float32(1.0) / np.float32(2.0 * SIGMA * SIGMA * Nb * Nb)
            R[3 * s + 0] = c
            R[3 * s + 1] = np.float32(-2.0) * c * o
            R[3 * s + 2] = c * o * o
            row0 += sum(tiles)
        in_maps.append({"a": A, "r": R, "lmat": lmat})
    return in_maps


def _build_bass(slots, reps: int = 1):
    import concourse.bacc as bacc
    import concourse.mybir as mybir
    from concourse.tile import TileContext

    fp32 = mybir.dt.float32
    n_slots = len(slots)
    tot_rows = sum(sum(tiles) for _, tiles, _ in slots)
    n_units = sum(
        len(tiles) * (-(-T_slot // CHUNK)) for _, tiles, T_slot in slots
    )
    assert n_units <= PART, f"too many units for accumulator: {n_units}"

    nc = bacc.Bacc(
        "TRN2", target_bir_lowering=False, debug=False, num_devices=N_CORES
    )
    a_d = nc.dram_tensor("a", [tot_rows, T_MAX], fp32, kind="ExternalInput")
    r_d = nc.dram_tensor("r", [n_slots * 3, T_MAX], fp32, kind="ExternalInput")
    l_d = nc.dram_tensor("lmat", [3, N_MAX], fp32, kind="ExternalInput")
    oacc_d = nc.dram_tensor("out_acc", [PART, PART], fp32, kind="ExternalOutput")

    with TileContext(nc) as tc:
        with (
            tc.tile_pool(name="const", bufs=1) as constp,
            tc.tile_pool(name="apool", bufs=5) as apool,
            tc.tile_pool(name="rpool", bufs=3) as rpool,
            tc.tile_pool(name="epool", bufs=6) as epool,
            tc.tile_pool(name="mpool", bufs=3) as mpool,
            tc.tile_pool(name="accp", bufs=1) as accp,
            tc.tile_pool(name="sqp", bufs=7, space="PSUM") as sqp,
        ):
            l_sb = constp.tile([3, N_MAX], fp32, tag="lmat")
            nc.sync.dma_start(out=l_sb[:], in_=l_d.ap()[:])
            acc = accp.tile([PART, PART], fp32, tag="acc")
            nc.vector.memset(acc[:], 0.0)

            for _rep in range(reps):
                unit = 0
                row0 = 0
                for s, (_, tiles, T_slot) in enumerate(slots):
                    r_sb = rpool.tile([3, T_MAX], fp32, tag="r")
                    nc.sync.dma_start(
                        out=r_sb[:, :T_slot], in_=r_d.ap()[3 * s : 3 * s + 3, :T_slot]
                    )
                    for rtile, rows in enumerate(tiles):
                        a_sb = apool.tile([PART, T_MAX], fp32, tag="a")
                        nc.sync.dma_start(
                            out=a_sb[:rows, :T_slot],
                            in_=a_d.ap()[
                                row0 + rtile * PART : row0 + rtile * PART + rows,
                                :T_slot,
                            ],
                        )
                        for c0 in range(0, T_slot, CHUNK):
                            w = min(CHUNK, T_slot - c0)
                            sq = sqp.tile([PART, CHUNK], fp32, tag="sq")
                            nc.tensor.matmul(
                                sq[:rows, :w],
                                l_sb[:, rtile * PART : rtile * PART + rows],
                                r_sb[:, c0 : c0 + w],
                                start=True,
                                stop=True,
                                skip_group_check=True,
                            )
                            e = epool.tile([PART, CHUNK], fp32, tag="e")
                            nc.scalar.activation(
                                e[:rows, :w],
                                sq[:rows, :w],
                                mybir.ActivationFunctionType.Exp,
                                bias=0.0,
                                scale=-1.0,
                            )
                            m = mpool.tile([PART, CHUNK], fp32, tag="m")
                            # m = (e - 1) * a ; acc column = row sums of m
                            nc.vector.scalar_tensor_tensor(
                                out=m[:rows, :w],
                                in0=e[:rows, :w],
                                scalar=1.0,
                                in1=a_sb[:rows, c0 : c0 + w],
                                op0=mybir.AluOpType.subtract,
                                op1=mybir.AluOpType.mult,
                                accum_out=acc[:rows, unit : unit + 1],
                            )
                            unit += 1
                    row0 += sum(tiles)
            nc.sync.dma_start(out=oacc_d.ap()[:], in_=acc[:])

    nc.compile()
    return nc


def _reduce_outputs(results):
    tot = 0.0
    for res in results:
        tot += np.asarray(res["out_acc"], dtype=np.float64).sum()
    loss = -tot / float(B * N_MAX * T_MAX)
    return np.array(loss, dtype=np.float32)


def kernel(alignments, input_lengths, target_lengths):
    from concourse.bass_utils import run_bass_kernel_spmd

    slots = _plan(input_lengths, target_lengths)
    in_maps = _host_inputs(alignments, input_lengths, target_lengths, slots)
    nc = _build_bass(slots, reps=1)
    out = run_bass_kernel_spmd(nc, in_maps, core_ids=list(range(N_CORES)))
    return _reduce_outputs(out.results)


if __name__ == "__main__":
    rng = np.random.default_rng(0)
    al = rng.random((B, N_MAX, T_MAX), dtype=np.float32)
    il = rng.integers(N_MAX // 2, N_MAX + 1, size=B).astype(np.int32)
    tl = rng.integers(T_MAX // 2, T_MAX + 1, size=B).astype(np.int32)
    print(kernel(alignments=al, input_lengths=il, target_lengths=tl))



# revision 5
# speedup vs baseline: 2.5062x; 2.5062x over previous
"""GuidedAttentionLoss on 8 Trainium2 NeuronCores (Bass/Tile).

loss = mean(guide * a^T) over [B=64, T=2048, N=512], where
  guide[b,t,k] = (1 - exp(-((k - floor(N_b/T_b * t))/N_b)^2 / (2*sigma^2)))
                 for t < T_b, k < N_b; 0 elsewhere.

Strategy (pure data parallel, 8 batches per core), v2:
  * Key identity: o_t = floor(N_b/T_b * t) takes only N_b distinct values,
    each over a run of consecutive t's (run lengths ~T/N).  So
      sum_t (e[k,o_t]-1) a[k,t] = sum_{o,j} (e[k,o]-1) P[k, j, o]
    where P[k, j, o] = a[k, t(o,j)] is a host-side column permutation of a
    into W "slabs" of width N (zero padded).  exp work shrinks by ~T/N.
  * e[k,o] = exp(-c_b (k-o)^2) computed by ScalarE directly from a constant
    integer (k-o)^2 table (bf16) with per-partition AP scale = -c_b.  No
    matmul, no PE, no fp32 4-cycles/row penalty, no cancellation.
  * a is staged as float8_e4m3 (loss tol 2e-2; RTN quantization noise
    averages out over 38M elements): 4x less DMA than fp32.
  * One fused DVE scalar_tensor_tensor per slot: (e-1)*P with a stride-0
    broadcast AP replaying each e row W times, accum_out giving per-
    partition sums.  All operands SBUF -> 2x DVE mode even at fp8.
  * Host: loss = -sum(acc)/ (B*N*T) in f64.
  * 64 batches dealt into 8 slots x 8 cores (swap hill-climb minimizing
    sum 128*ntiles*W*N = DMA bytes = DVE cycles) so one SPMD program fits
    all cores with small padding.
"""

import numpy as np
import ml_dtypes

B, N_MAX, T_MAX = 64, 512, 2048
SIGMA = 0.4
N_CORES = 8
PART = 128
NTILES_MAX = 4  # ceil(N_MAX / PART)
F8 = ml_dtypes.float8_e4m3
BF16 = ml_dtypes.bfloat16


def _runs(Nb, Tb):
    """Per-t offset o_t (exact reference fp32 math), slab index j_t, W."""
    t = np.arange(Tb, dtype=np.float32)
    ratio = np.float32(Nb) / np.float32(Tb)
    o = np.floor(ratio * t).astype(np.int64)
    starts = np.empty(Tb, dtype=bool)
    starts[0] = True
    starts[1:] = o[1:] != o[:-1]
    first = np.zeros(int(o[-1]) + 1, dtype=np.int64)
    first[o[starts]] = np.nonzero(starts)[0]
    j = np.arange(Tb, dtype=np.int64) - first[o]
    return o, j, int(j.max()) + 1


def _slot_shape(Ns, Ws):
    ntiles = -(-Ns // PART)
    return ntiles, ntiles * Ws * Ns  # (row tiles, per-core free length)


def _plan(input_lengths, target_lengths):
    """Assign the B batches to (slot, core) minimizing total padded volume.

    Returns (slots, LEN): slots is a list of dicts with idxs (batch id per
    core), N_s, W_s, ntiles, free, base.  Cost per slot = 128*ntiles*W*N
    bytes (fp8), which is proportional to both DMA bytes and DVE cycles.
    """
    Ns = np.asarray(input_lengths, dtype=np.int64)
    Ts = np.asarray(target_lengths, dtype=np.int64)
    assert Ns.shape == (B,) and Ts.shape == (B,)
    Ws = np.array([_runs(int(Ns[b]), int(Ts[b]))[2] for b in range(B)],
                  dtype=np.int64)
    n_slots = B // N_CORES

    def slot_cost(g):
        N_s = int(max(Ns[i] for i in g))
        W_s = int(max(Ws[i] for i in g))
        ntiles = -(-N_s // PART)
        return PART * ntiles * W_s * N_s

    def sweep(groups):
        improved = True
        while improved:
            improved = False
            for s1 in range(n_slots):
                for s2 in range(s1 + 1, n_slots):
                    g1, g2 = groups[s1], groups[s2]
                    for i1 in range(N_CORES):
                        for i2 in range(N_CORES):
                            c0 = slot_cost(g1) + slot_cost(g2)
                            g1[i1], g2[i2] = g2[i2], g1[i1]
                            if slot_cost(g1) + slot_cost(g2) < c0:
                                improved = True
                            else:
                                g1[i1], g2[i2] = g2[i2], g1[i1]
        return groups

    rng = np.random.default_rng(0)
    best_cost, groups = None, None
    for trial in range(12):
        if trial == 0:
            order = np.argsort(-(Ws * 10000 + Ns))  # W major, N minor
        elif trial == 1:
            order = np.argsort(-(Ns * 100 + Ws))
        elif trial == 2:
            order = np.argsort(-(Ns * Ns * Ws))
        else:
            order = rng.permutation(B)
        cand = sweep(
            [list(order[s * N_CORES: (s + 1) * N_CORES]) for s in range(n_slots)]
        )
        c = sum(slot_cost(g) for g in cand)
        if best_cost is None or c < best_cost:
            best_cost, groups = c, [list(g) for g in cand]

    slots, base = [], 0
    for g in groups:
        idxs = np.array([int(i) for i in g])
        N_s = int(Ns[idxs].max())
        W_s = int(Ws[idxs].max())
        ntiles, free = _slot_shape(N_s, W_s)
        slots.append(dict(idxs=idxs, N_s=N_s, W_s=W_s, ntiles=ntiles,
                          free=free, base=base))
        base += free
    return slots, base


def _host_inputs(alignments, input_lengths, target_lengths, slots, LEN):
    """Per-core input dicts for run_bass_kernel_spmd."""
    alignments = np.asarray(alignments)
    n_slots = len(slots)

    # Constant (k-o)^2 table, shared by all cores: D2[p, rt*N_MAX + o].
    p = np.arange(PART, dtype=np.float32)[:, None]
    o = np.arange(N_MAX, dtype=np.float32)[None, :]
    d2 = np.concatenate(
        [((rt * PART + p) - o) ** 2 for rt in range(NTILES_MAX)], axis=1
    ).astype(BF16)

    in_maps = []
    for core in range(N_CORES):
        blob = np.zeros((PART, LEN), dtype=F8)
        scales = np.zeros((PART, n_slots), dtype=np.float32)
        for s, sl in enumerate(slots):
            b = int(sl["idxs"][core])
            Nb = int(input_lengths[b])
            Tb = int(target_lengths[b])
            N_s, W_s = sl["N_s"], sl["W_s"]
            o_t, j_t, _ = _runs(Nb, Tb)
            cols = j_t * N_s + o_t  # within a row tile
            a8 = alignments[b, :Nb, :Tb].astype(F8)
            for rt in range(sl["ntiles"]):
                lo = rt * PART
                hi = min(lo + PART, Nb)
                if hi <= lo:
                    break
                blob[0: hi - lo, sl["base"] + rt * (W_s * N_s) + cols] = a8[lo:hi]
            scales[:, s] = np.float32(-1.0) / np.float32(
                2.0 * SIGMA * SIGMA * Nb * Nb
            )
        in_maps.append({"blob": blob, "d2": d2, "scales": scales})
    return in_maps


def _build_bass(slots, reps: int = 1):
    import concourse.bacc as bacc
    import concourse.mybir as mybir
    from concourse.tile import TileContext

    fp32 = mybir.dt.float32
    bf16 = mybir.dt.bfloat16
    f8 = mybir.dt.float8e4
    n_slots = len(slots)
    LEN = sum(sl["free"] for sl in slots)
    max_free = max(sl["free"] for sl in slots)
    max_e = max(sl["ntiles"] * sl["N_s"] for sl in slots)
    max_m = max(sl["W_s"] * sl["N_s"] for sl in slots)
    n_units = sum(sl["ntiles"] for sl in slots)

    nc = bacc.Bacc(
        "TRN2", target_bir_lowering=False, debug=False, num_devices=N_CORES
    )
    blob_d = nc.dram_tensor("blob", [PART, LEN], f8, kind="ExternalInput")
    d2_d = nc.dram_tensor("d2", [PART, NTILES_MAX * N_MAX], bf16,
                          kind="ExternalInput")
    sc_d = nc.dram_tensor("scales", [PART, n_slots], fp32, kind="ExternalInput")
    oacc_d = nc.dram_tensor("out_acc", [PART, n_units], fp32,
                            kind="ExternalOutput")

    with TileContext(nc) as tc:
        with (
            tc.tile_pool(name="const", bufs=1) as constp,
            tc.tile_pool(name="blobp", bufs=3) as blobp,
            tc.tile_pool(name="epool", bufs=2) as epool,
            tc.tile_pool(name="mpool", bufs=2) as mpool,
            tc.tile_pool(name="accp", bufs=1) as accp,
        ):
            d2_sb = constp.tile([PART, NTILES_MAX * N_MAX], bf16, tag="d2")
            nc.sync.dma_start(out=d2_sb[:], in_=d2_d.ap()[:])
            sc_sb = constp.tile([PART, n_slots], fp32, tag="sc")
            nc.sync.dma_start(out=sc_sb[:], in_=sc_d.ap()[:])
            acc = accp.tile([PART, n_units], fp32, tag="acc")
            nc.vector.memset(acc[:], 0.0)

            for _rep in range(reps):
                u = 0
                for s, sl in enumerate(slots):
                    nt, W, N = sl["ntiles"], sl["W_s"], sl["N_s"]
                    free = sl["free"]
                    blob_t = blobp.tile([PART, max_free], f8, tag="blob")
                    nc.sync.dma_start(
                        out=blob_t[:, :free],
                        in_=blob_d.ap()[:, sl["base"]: sl["base"] + free],
                    )
                    e_t = epool.tile([PART, max_e], bf16, tag="e")
                    d2_in = d2_sb[:, : nt * N_MAX].rearrange(
                        "p (r o) -> p r o", r=nt
                    )[:, :, 0:N]
                    e_out = e_t[:, : nt * N].rearrange("p (r o) -> p r o", r=nt)
                    nc.scalar.activation(
                        e_out, d2_in, mybir.ActivationFunctionType.Exp,
                        bias=0.0, scale=sc_sb[:, s: s + 1],
                    )
                    for rt in range(nt):
                        m_t = mpool.tile([PART, max_m], bf16, tag="m")
                        in0 = (
                            e_t[:, rt * N: (rt + 1) * N]
                            .unsqueeze(1)
                            .broadcast_to([PART, W, N])
                        )
                        in1 = blob_t[:, rt * W * N: (rt + 1) * W * N].rearrange(
                            "p (w o) -> p w o", w=W
                        )
                        m_out = m_t[:, : W * N].rearrange(
                            "p (w o) -> p w o", w=W
                        )
                        nc.vector.scalar_tensor_tensor(
                            out=m_out, in0=in0, scalar=1.0, in1=in1,
                            op0=mybir.AluOpType.subtract,
                            op1=mybir.AluOpType.mult,
                            accum_out=acc[:, u: u + 1],
                        )
                        u += 1
            nc.sync.dma_start(out=oacc_d.ap()[:], in_=acc[:])

    nc.compile()
    return nc


def _reduce_outputs(results):
    tot = 0.0
    for res in results:
        tot += np.asarray(res["out_acc"], dtype=np.float64).sum()
    loss = -tot / float(B * N_MAX * T_MAX)
    return np.array(loss, dtype=np.float32)


def kernel(alignments, input_lengths, target_lengths):
    from concourse.bass_utils import run_bass_kernel_spmd

    slots, LEN = _plan(input_lengths, target_lengths)
    in_maps = _host_inputs(alignments, input_lengths, target_lengths, slots, LEN)
    nc = _build_bass(slots, reps=1)
    out = run_bass_kernel_spmd(nc, in_maps, core_ids=list(range(N_CORES)))
    return _reduce_outputs(out.results)


if __name__ == "__main__":
    rng = np.random.default_rng(0)
    al = rng.random((B, N_MAX, T_MAX), dtype=np.float32)
    il = rng.integers(N_MAX // 2, N_MAX + 1, size=B).astype(np.int32)
    tl = rng.integers(T_MAX // 2, T_MAX + 1, size=B).astype(np.int32)
    print(kernel(alignments=al, input_lengths=il, target_lengths=tl))


# revision 6
# speedup vs baseline: 3.2998x; 1.3167x over previous
"""GuidedAttentionLoss on 8 Trainium2 NeuronCores (Bass/Tile).

loss = mean(guide * a^T) over [B=64, T=2048, N=512], where
  guide[b,t,k] = (1 - exp(-((k - floor(N_b/T_b * t))/N_b)^2 / (2*sigma^2)))
                 for t < T_b, k < N_b; 0 elsewhere.

Strategy (pure data parallel, 8 batches per core), v3:
  * Key identity: o_t = floor(N_b/T_b * t) takes only N_b distinct values,
    each over a run of consecutive t's (run lengths ~T/N).  So
      sum_t (e[k,o_t]-1) a[k,t] = sum_{o,j} (e[k,o]-1) P[k, j, o]
    where P[k, j, o] = a[k, t(o,j)] is a host-side column permutation of a
    into W "slabs" of width N (zero padded).  exp work shrinks by ~T/N.
  * e[k,o] = exp(-c_b (k-o)^2) computed by ScalarE directly from a constant
    integer (k-o)^2 table (bf16) with per-partition AP scale = -c_b.  No
    matmul, no PE, no fp32 4-cycles/row penalty, no cancellation.
  * a is staged as float8_e4m3 or bfloat16 per slot (loss tol 2e-2; RTN
    quantization noise averages out over 38M elements).  DVE runs the fused
    (e-1)*P scalar_tensor_tensor at 2x (all-SBUF) for fp8 and 4x (2-byte
    packed + all-SBUF) for bf16; DMA is 1B/elem for fp8, 2B for bf16.  The
    fp8/bf16 split is chosen to balance DVE time against DMA time.
  * stt per (slot, row-tile): in0 = e row broadcast W times via stride-0 AP
    (3D, BIR limit), accum_out per unit; host sums valid lanes in f64.
  * DMAs are row-trimmed ([rows, W*N], skipping zero rows of the last row
    tile); garbage in the untouched partitions stays lane-isolated in the
    accumulator and is masked out on the host.
  * 64 batches dealt into 8 slots x 8 cores by simulated annealing + sweep
    minimizing sum ntiles*W*N (DVE cols, also ~DMA bytes).
"""

import numpy as np
import ml_dtypes

B, N_MAX, T_MAX = 64, 512, 2048
SIGMA = 0.4
N_CORES = 8
PART = 128
NTILES_MAX = 4  # ceil(N_MAX / PART)
F8 = ml_dtypes.float8_e4m3
BF16 = ml_dtypes.bfloat16

# engine model (per core): ns per free-dim column / per byte
_DVE_NS = 1.0416666  # 1x; 2x all-SBUF, 4x 2-byte packed all-SBUF
_ACT_NS = 0.8333333
_DMA_BPNS = 360.0  # bytes per ns, all 16 engines


def _runs(Nb, Tb):
    """Per-t offset o_t (exact reference fp32 math), slab index j_t, W."""
    t = np.arange(Tb, dtype=np.float32)
    ratio = np.float32(Nb) / np.float32(Tb)
    o = np.floor(ratio * t).astype(np.int64)
    starts = np.empty(Tb, dtype=bool)
    starts[0] = True
    starts[1:] = o[1:] != o[:-1]
    first = np.zeros(int(o[-1]) + 1, dtype=np.int64)
    first[o[starts]] = np.nonzero(starts)[0]
    j = np.arange(Tb, dtype=np.int64) - first[o]
    return o, j, int(j.max()) + 1


def _plan(input_lengths, target_lengths):
    """Assign batches to (slot, core); pick per-slot dtype (fp8/bf16).

    Returns list of slot dicts: idxs, N_s, W_s, ntiles, free, base, bf16.
    base indexes into the dtype's own blob.
    """
    Ns = np.asarray(input_lengths, dtype=np.int64)
    Ts = np.asarray(target_lengths, dtype=np.int64)
    assert Ns.shape == (B,) and Ts.shape == (B,)
    Ws = np.array([_runs(int(Ns[b]), int(Ts[b]))[2] for b in range(B)],
                  dtype=np.int64)
    n_slots = B // N_CORES

    def slot_cost(g):
        N_s = int(max(Ns[i] for i in g))
        W_s = int(max(Ws[i] for i in g))
        return (-(-N_s // PART)) * W_s * N_s  # DVE free-dim columns

    rng = np.random.default_rng(0)

    def sweep(groups):
        improved = True
        while improved:
            improved = False
            for s1 in range(n_slots):
                for s2 in range(s1 + 1, n_slots):
                    g1, g2 = groups[s1], groups[s2]
                    for i1 in range(N_CORES):
                        for i2 in range(N_CORES):
                            c0 = slot_cost(g1) + slot_cost(g2)
                            g1[i1], g2[i2] = g2[i2], g1[i1]
                            if slot_cost(g1) + slot_cost(g2) < c0:
                                improved = True
                            else:
                                g1[i1], g2[i2] = g2[i2], g1[i1]
        return groups

    def anneal(groups, iters=120000, T0=400.0, T1=0.5):
        groups = [list(g) for g in groups]
        costs = [slot_cost(g) for g in groups]
        cur = sum(costs)
        best, bestg = cur, [list(g) for g in groups]
        log_ratio = np.log(T1 / T0)
        u_rand = rng.random(iters)
        idx = rng.integers(0, 8, size=(iters, 4))
        for it in range(iters):
            s1, s2, i1, i2 = idx[it]
            if s1 == s2:
                continue
            T = T0 * np.exp(log_ratio * it / iters)
            g1, g2 = groups[s1], groups[s2]
            g1[i1], g2[i2] = g2[i2], g1[i1]
            c1, c2 = slot_cost(g1), slot_cost(g2)
            d = c1 + c2 - costs[s1] - costs[s2]
            if d <= 0 or u_rand[it] < np.exp(-d / T):
                costs[s1], costs[s2] = c1, c2
                cur += d
                if cur < best:
                    best, bestg = cur, [list(g) for g in groups]
            else:
                g1[i1], g2[i2] = g2[i2], g1[i1]
        return best, bestg

    order = np.argsort(-(Ws * 10000 + Ns))
    g0 = [list(order[s * N_CORES: (s + 1) * N_CORES]) for s in range(n_slots)]
    best_cost, best_g = anneal(g0)
    for _ in range(2):
        perm = rng.permutation(B)
        c, g = anneal([list(perm[s * N_CORES: (s + 1) * N_CORES])
                       for s in range(n_slots)])
        if c < best_cost:
            best_cost, best_g = c, g
    best_g = sweep([list(g) for g in best_g])

    raw = []
    for g in best_g:
        idxs = np.array([int(i) for i in g])
        N_s = int(Ns[idxs].max())
        W_s = int(Ws[idxs].max())
        ntiles = -(-N_s // PART)
        raw.append(dict(idxs=idxs, N_s=N_s, W_s=W_s, ntiles=ntiles,
                        free=ntiles * W_s * N_s))

    # dtype split: brute-force the subset of bf16 slots that minimizes
    # max(DVE, DMA) under the cost model.
    v = [sl["ntiles"] * sl["W_s"] * sl["N_s"] for sl in raw]  # DVE cols
    dbytes = [sl["N_s"] * sl["W_s"] * sl["N_s"] for sl in raw]  # trimmed B
    act = sum(sl["ntiles"] * sl["N_s"] for sl in raw) * _ACT_NS
    best_t, best_mask = None, 0
    for mask in range(1 << n_slots):
        dve = sum(v[s] * (_DVE_NS * (0.25 if mask >> s & 1 else 0.5))
                  for s in range(n_slots))
        dma = sum(dbytes[s] * (2 if mask >> s & 1 else 1)
                  for s in range(n_slots)) / _DMA_BPNS
        t = max(dve, dma, act)
        if best_t is None or t < best_t:
            best_t, best_mask = t, mask

    slots, base8, base16 = [], 0, 0
    for s, sl in enumerate(raw):
        bf16 = bool(best_mask >> s & 1)
        sl["bf16"] = bf16
        sl["base"] = base16 if bf16 else base8
        if bf16:
            base16 += sl["free"]
        else:
            base8 += sl["free"]
        slots.append(sl)
    return slots, base8, base16


def _host_inputs(alignments, input_lengths, target_lengths, slots,
                 len8, len16):
    """Per-core input dicts for run_bass_kernel_spmd."""
    alignments = np.asarray(alignments)
    n_slots = len(slots)

    # Constant (k-o)^2 table, shared by all cores: D2[p, rt*N_MAX + o].
    p = np.arange(PART, dtype=np.float32)[:, None]
    o = np.arange(N_MAX, dtype=np.float32)[None, :]
    d2 = np.concatenate(
        [((rt * PART + p) - o) ** 2 for rt in range(NTILES_MAX)], axis=1
    ).astype(BF16)

    in_maps = []
    for core in range(N_CORES):
        blob8 = np.zeros((PART, max(len8, 1)), dtype=F8)
        blob16 = np.zeros((PART, max(len16, 1)), dtype=BF16)
        scales = np.zeros((PART, n_slots), dtype=np.float32)
        for s, sl in enumerate(slots):
            b = int(sl["idxs"][core])
            Nb = int(input_lengths[b])
            Tb = int(target_lengths[b])
            N_s, W_s = sl["N_s"], sl["W_s"]
            o_t, j_t, _ = _runs(Nb, Tb)
            cols = j_t * N_s + o_t  # within a row tile
            blob = blob16 if sl["bf16"] else blob8
            a_cast = alignments[b, :Nb, :Tb].astype(blob.dtype)
            for rt in range(sl["ntiles"]):
                lo = rt * PART
                hi = min(lo + PART, Nb)
                if hi <= lo:
                    break
                blob[0: hi - lo, sl["base"] + rt * (W_s * N_s) + cols] = \
                    a_cast[lo:hi]
            scales[:, s] = np.float32(-1.0) / np.float32(
                2.0 * SIGMA * SIGMA * Nb * Nb
            )
        in_maps.append({"blob8": blob8, "blob16": blob16, "d2": d2,
                        "scales": scales})
    return in_maps


def _build_bass(slots, reps: int = 1):
    import concourse.bacc as bacc
    import concourse.mybir as mybir
    from concourse.tile import TileContext

    fp32 = mybir.dt.float32
    bf16 = mybir.dt.bfloat16
    f8 = mybir.dt.float8e4
    n_slots = len(slots)
    len8 = sum(sl["free"] for sl in slots if not sl["bf16"])
    len16 = sum(sl["free"] for sl in slots if sl["bf16"])
    max_f8 = max([sl["free"] for sl in slots if not sl["bf16"]] or [1])
    max_f16 = max([sl["free"] for sl in slots if sl["bf16"]] or [1])
    max_e = max(sl["ntiles"] * sl["N_s"] for sl in slots)
    max_m = max(sl["W_s"] * sl["N_s"] for sl in slots)
    n_units = sum(sl["ntiles"] for sl in slots)

    nc = bacc.Bacc(
        "TRN2", target_bir_lowering=False, debug=False, num_devices=N_CORES
    )
    blob8_d = nc.dram_tensor("blob8", [PART, max(len8, 1)], f8,
                             kind="ExternalInput")
    blob16_d = nc.dram_tensor("blob16", [PART, max(len16, 1)], bf16,
                              kind="ExternalInput")
    d2_d = nc.dram_tensor("d2", [PART, NTILES_MAX * N_MAX], bf16,
                          kind="ExternalInput")
    sc_d = nc.dram_tensor("scales", [PART, n_slots], fp32, kind="ExternalInput")
    oacc_d = nc.dram_tensor("out_acc", [PART, n_units], fp32,
                            kind="ExternalOutput")

    with TileContext(nc) as tc:
        with (
            tc.tile_pool(name="const", bufs=1) as constp,
            tc.tile_pool(name="blob8p", bufs=3) as blob8p,
            tc.tile_pool(name="blob16p", bufs=3) as blob16p,
            tc.tile_pool(name="epool", bufs=2) as epool,
            tc.tile_pool(name="mpool", bufs=2) as mpool,
            tc.tile_pool(name="accp", bufs=1) as accp,
        ):
            d2_sb = constp.tile([PART, NTILES_MAX * N_MAX], bf16, tag="d2")
            nc.sync.dma_start(out=d2_sb[:], in_=d2_d.ap()[:])
            sc_sb = constp.tile([PART, n_slots], fp32, tag="sc")
            nc.sync.dma_start(out=sc_sb[:], in_=sc_d.ap()[:])
            acc = accp.tile([PART, n_units], fp32, tag="acc")
            nc.vector.memset(acc[:], 0.0)

            for _rep in range(reps):
                u = 0
                for s, sl in enumerate(slots):
                    nt, W, N = sl["ntiles"], sl["W_s"], sl["N_s"]
                    if sl["bf16"]:
                        blob_t = blob16p.tile([PART, max_f16], bf16, tag="b16")
                        blob_src = blob16_d
                    else:
                        blob_t = blob8p.tile([PART, max_f8], f8, tag="b8")
                        blob_src = blob8_d
                    for rt in range(nt):
                        rows = min(PART, N - rt * PART)
                        nc.sync.dma_start(
                            out=blob_t[0:rows, rt * W * N: (rt + 1) * W * N],
                            in_=blob_src.ap()[
                                0:rows,
                                sl["base"] + rt * W * N:
                                sl["base"] + (rt + 1) * W * N,
                            ],
                        )
                    e_t = epool.tile([PART, max_e], bf16, tag="e")
                    d2_in = d2_sb[:, : nt * N_MAX].rearrange(
                        "p (r o) -> p r o", r=nt
                    )[:, :, 0:N]
                    e_out = e_t[:, : nt * N].rearrange("p (r o) -> p r o", r=nt)
                    nc.scalar.activation(
                        e_out, d2_in, mybir.ActivationFunctionType.Exp,
                        bias=0.0, scale=sc_sb[:, s: s + 1],
                    )
                    for rt in range(nt):
                        m_t = mpool.tile([PART, max_m], bf16, tag="m")
                        in0 = (
                            e_t[:, rt * N: (rt + 1) * N]
                            .unsqueeze(1)
                            .broadcast_to([PART, W, N])
                        )
                        in1 = blob_t[:, rt * W * N: (rt + 1) * W * N].rearrange(
                            "p (w o) -> p w o", w=W
                        )
                        m_out = m_t[:, : W * N].rearrange(
                            "p (w o) -> p w o", w=W
                        )
                        nc.vector.scalar_tensor_tensor(
                            out=m_out, in0=in0, scalar=1.0, in1=in1,
                            op0=mybir.AluOpType.subtract,
                            op1=mybir.AluOpType.mult,
                            accum_out=acc[:, u: u + 1],
                        )
                        u += 1
            nc.sync.dma_start(out=oacc_d.ap()[:], in_=acc[:])

    nc.compile()
    return nc


def _reduce_outputs(results, slots):
    tot = 0.0
    for res in results:
        acc = np.asarray(res["out_acc"], dtype=np.float64)
        u = 0
        for sl in slots:
            N = sl["N_s"]
            for rt in range(sl["ntiles"]):
                rows = min(PART, N - rt * PART)
                tot += acc[0:rows, u].sum()
                u += 1
    loss = -tot / float(B * N_MAX * T_MAX)
    return np.array(loss, dtype=np.float32)


def kernel(alignments, input_lengths, target_lengths):
    from concourse.bass_utils import run_bass_kernel_spmd

    slots, len8, len16 = _plan(input_lengths, target_lengths)
    in_maps = _host_inputs(alignments, input_lengths, target_lengths, slots,
                           len8, len16)
    nc = _build_bass(slots, reps=1)
    out = run_bass_kernel_spmd(nc, in_maps, core_ids=list(range(N_CORES)))
    return _reduce_outputs(out.results, slots)


if __name__ == "__main__":
    rng = np.random.default_rng(0)
    al = rng.random((B, N_MAX, T_MAX), dtype=np.float32)
    il = rng.integers(N_MAX // 2, N_MAX + 1, size=B).astype(np.int32)
    tl = rng.integers(T_MAX // 2, T_MAX + 1, size=B).astype(np.int32)
    print(kernel(alignments=al, input_lengths=il, target_lengths=tl))


# revision 9
# speedup vs baseline: 5.4108x; 1.6397x over previous
"""GuidedAttentionLoss on 8 Trainium2 NeuronCores (Bass/Tile).

loss = mean(guide * a^T) over [B=64, T=2048, N=512], where
  guide[b,t,k] = (1 - exp(-((k - floor(N_b/T_b * t))/N_b)^2 / (2*sigma^2)))
                 for t < T_b, k < N_b; 0 elsewhere.

Strategy (pure data parallel, 8 batches per core), v3:
  * Key identity: o_t = floor(N_b/T_b * t) takes only N_b distinct values,
    each over a run of consecutive t's (run lengths ~T/N).  So
      sum_t (e[k,o_t]-1) a[k,t] = sum_{o,j} (e[k,o]-1) P[k, j, o]
    where P[k, j, o] = a[k, t(o,j)] is a host-side column permutation of a
    into W "slabs" of width N (zero padded).  exp work shrinks by ~T/N.
  * e[k,o] = exp(-c_b (k-o)^2) computed by ScalarE directly from a constant
    integer (k-o)^2 table (bf16) with per-partition AP scale = -c_b.  No
    matmul, no PE, no fp32 4-cycles/row penalty, no cancellation.
  * a is staged as float8_e4m3 or bfloat16 per slot (loss tol 2e-2; RTN
    quantization noise averages out over 38M elements).  DVE runs the fused
    (e-1)*P scalar_tensor_tensor at 2x (all-SBUF) for fp8 and 4x (2-byte
    packed + all-SBUF) for bf16; DMA is 1B/elem for fp8, 2B for bf16.  The
    fp8/bf16 split is chosen to balance DVE time against DMA time.
  * stt per (slot, row-tile): in0 = e row broadcast W times via stride-0 AP
    (3D, BIR limit), accum_out per unit; host sums valid lanes in f64.
  * DMAs are row-trimmed ([rows, W*N], skipping zero rows of the last row
    tile); garbage in the untouched partitions stays lane-isolated in the
    accumulator and is masked out on the host.
  * 64 batches dealt into 8 slots x 8 cores by simulated annealing + sweep
    minimizing sum ntiles*W*N (DVE cols, also ~DMA bytes).
"""

import os

import numpy as np
import ml_dtypes

# experiment toggles (harness uses defaults)
_FORCE_MASK = os.environ.get("K_FORCE_MASK")  # int: per-slot bf16 bitmask
_CAST_DMA = os.environ.get("K_CAST_DMA") == "1"  # fp8 DRAM -> bf16 SBUF DMA

B, N_MAX, T_MAX = 64, 512, 2048
SIGMA = 0.4
N_CORES = 8
PART = 128
NTILES_MAX = 4  # ceil(N_MAX / PART)
F8 = ml_dtypes.float8_e4m3
BF16 = ml_dtypes.bfloat16

# engine model (per core): ns per free-dim column / per byte
_DVE_NS = 1.0416666  # 1x; 2x all-SBUF, 4x 2-byte packed all-SBUF
_ACT_NS = 0.8333333
_DMA_BPNS = 360.0  # bytes per ns, all 16 engines


def _runs(Nb, Tb):
    """Per-t offset o_t (exact reference fp32 math), slab index j_t, W."""
    t = np.arange(Tb, dtype=np.float32)
    ratio = np.float32(Nb) / np.float32(Tb)
    o = np.floor(ratio * t).astype(np.int64)
    starts = np.empty(Tb, dtype=bool)
    starts[0] = True
    starts[1:] = o[1:] != o[:-1]
    first = np.zeros(int(o[-1]) + 1, dtype=np.int64)
    first[o[starts]] = np.nonzero(starts)[0]
    j = np.arange(Tb, dtype=np.int64) - first[o]
    return o, j, int(j.max()) + 1


def _plan(input_lengths, target_lengths):
    """Assign batches to (slot, core); pick per-slot dtype (fp8/bf16).

    Returns list of slot dicts: idxs, N_s, W_s, ntiles, free, base, bf16.
    base indexes into the dtype's own blob.
    """
    Ns = np.asarray(input_lengths, dtype=np.int64)
    Ts = np.asarray(target_lengths, dtype=np.int64)
    assert Ns.shape == (B,) and Ts.shape == (B,)
    Ws = np.array([_runs(int(Ns[b]), int(Ts[b]))[2] for b in range(B)],
                  dtype=np.int64)
    n_slots = B // N_CORES

    def slot_cost(g):
        N_s = int(max(Ns[i] for i in g))
        W_s = int(max(Ws[i] for i in g))
        return (-(-N_s // PART)) * W_s * N_s  # DVE free-dim columns

    rng = np.random.default_rng(0)

    def sweep(groups):
        improved = True
        while improved:
            improved = False
            for s1 in range(n_slots):
                for s2 in range(s1 + 1, n_slots):
                    g1, g2 = groups[s1], groups[s2]
                    for i1 in range(N_CORES):
                        for i2 in range(N_CORES):
                            c0 = slot_cost(g1) + slot_cost(g2)
                            g1[i1], g2[i2] = g2[i2], g1[i1]
                            if slot_cost(g1) + slot_cost(g2) < c0:
                                improved = True
                            else:
                                g1[i1], g2[i2] = g2[i2], g1[i1]
        return groups

    def anneal(groups, iters=120000, T0=400.0, T1=0.5):
        groups = [list(g) for g in groups]
        costs = [slot_cost(g) for g in groups]
        cur = sum(costs)
        best, bestg = cur, [list(g) for g in groups]
        log_ratio = np.log(T1 / T0)
        u_rand = rng.random(iters)
        idx = rng.integers(0, 8, size=(iters, 4))
        for it in range(iters):
            s1, s2, i1, i2 = idx[it]
            if s1 == s2:
                continue
            T = T0 * np.exp(log_ratio * it / iters)
            g1, g2 = groups[s1], groups[s2]
            g1[i1], g2[i2] = g2[i2], g1[i1]
            c1, c2 = slot_cost(g1), slot_cost(g2)
            d = c1 + c2 - costs[s1] - costs[s2]
            if d <= 0 or u_rand[it] < np.exp(-d / T):
                costs[s1], costs[s2] = c1, c2
                cur += d
                if cur < best:
                    best, bestg = cur, [list(g) for g in groups]
            else:
                g1[i1], g2[i2] = g2[i2], g1[i1]
        return best, bestg

    order = np.argsort(-(Ws * 10000 + Ns))
    g0 = [list(order[s * N_CORES: (s + 1) * N_CORES]) for s in range(n_slots)]
    best_cost, best_g = anneal(g0)
    for _ in range(2):
        perm = rng.permutation(B)
        c, g = anneal([list(perm[s * N_CORES: (s + 1) * N_CORES])
                       for s in range(n_slots)])
        if c < best_cost:
            best_cost, best_g = c, g
    best_g = sweep([list(g) for g in best_g])

    raw = []
    for g in best_g:
        idxs = np.array([int(i) for i in g])
        N_s = int(Ns[idxs].max())
        W_s = int(Ws[idxs].max())
        ntiles = -(-N_s // PART)
        raw.append(dict(idxs=idxs, N_s=N_s, W_s=W_s, ntiles=ntiles,
                        free=ntiles * W_s * N_s))

    # dtype split: brute-force the subset of bf16 slots that minimizes
    # max(DVE, DMA) under the cost model.
    v = [sl["ntiles"] * sl["W_s"] * sl["N_s"] for sl in raw]  # DVE cols
    dbytes = [sl["N_s"] * sl["W_s"] * sl["N_s"] for sl in raw]  # trimmed B
    act = sum(sl["ntiles"] * sl["N_s"] for sl in raw) * _ACT_NS
    best_t, best_mask = None, 0
    for mask in range(1 << n_slots):
        dve = sum(v[s] * (_DVE_NS * (0.25 if mask >> s & 1 else 0.5))
                  for s in range(n_slots))
        dma = sum(dbytes[s] * (2 if mask >> s & 1 else 1)
                  for s in range(n_slots)) / _DMA_BPNS
        t = max(dve, dma, act)
        if best_t is None or t < best_t:
            best_t, best_mask = t, mask
    if _FORCE_MASK is not None:
        best_mask = int(_FORCE_MASK)
    if _CAST_DMA:
        best_mask = 0  # stage everything fp8; compute upcasts via DMA

    slots, base8, base16 = [], 0, 0
    for s, sl in enumerate(raw):
        bf16 = bool(best_mask >> s & 1)
        sl["bf16"] = bf16
        sl["base"] = base16 if bf16 else base8
        if bf16:
            base16 += sl["free"]
        else:
            base8 += sl["free"]
        slots.append(sl)
    return slots, base8, base16


def _host_inputs(alignments, input_lengths, target_lengths, slots,
                 len8, len16):
    """Per-core input dicts for run_bass_kernel_spmd."""
    alignments = np.asarray(alignments)
    n_slots = len(slots)

    # Constant (k-o)^2 table, shared by all cores: D2[p, rt*N_MAX + o].
    p = np.arange(PART, dtype=np.float32)[:, None]
    o = np.arange(N_MAX, dtype=np.float32)[None, :]
    d2 = np.concatenate(
        [((rt * PART + p) - o) ** 2 for rt in range(NTILES_MAX)], axis=1
    ).astype(BF16)

    in_maps = []
    for core in range(N_CORES):
        blob8 = np.zeros((PART, max(len8, 1)), dtype=F8)
        blob16 = np.zeros((PART, max(len16, 1)), dtype=BF16)
        scales = np.zeros((PART, n_slots), dtype=np.float32)
        for s, sl in enumerate(slots):
            b = int(sl["idxs"][core])
            Nb = int(input_lengths[b])
            Tb = int(target_lengths[b])
            N_s, W_s = sl["N_s"], sl["W_s"]
            o_t, j_t, _ = _runs(Nb, Tb)
            cols = j_t * N_s + o_t  # within a row tile
            blob = blob16 if sl["bf16"] else blob8
            a_cast = alignments[b, :Nb, :Tb].astype(blob.dtype)
            for rt in range(sl["ntiles"]):
                lo = rt * PART
                hi = min(lo + PART, Nb)
                if hi <= lo:
                    break
                blob[0: hi - lo, sl["base"] + rt * (W_s * N_s) + cols] = \
                    a_cast[lo:hi]
            scales[:, s] = np.float32(-1.0) / np.float32(
                2.0 * SIGMA * SIGMA * Nb * Nb
            )
        in_maps.append({"blob8": blob8, "blob16": blob16, "d2": d2,
                        "scales": scales})
    return in_maps


def _build_bass(slots, reps: int = 1):
    import concourse.bacc as bacc
    import concourse.mybir as mybir
    from concourse.tile import TileContext

    fp32 = mybir.dt.float32
    bf16 = mybir.dt.bfloat16
    f8 = mybir.dt.float8e4
    n_slots = len(slots)
    len8 = sum(sl["free"] for sl in slots if not sl["bf16"])
    len16 = sum(sl["free"] for sl in slots if sl["bf16"])
    max_f8 = max([sl["free"] for sl in slots if not sl["bf16"]] or [1])
    max_f16 = max([sl["free"] for sl in slots if sl["bf16"]] or [1])
    max_e = max(sl["ntiles"] * sl["N_s"] for sl in slots)
    max_m = max(sl["W_s"] * sl["N_s"] for sl in slots)
    n_units = sum(sl["ntiles"] for sl in slots)

    nc = bacc.Bacc(
        "TRN2", target_bir_lowering=False, debug=False, num_devices=N_CORES
    )
    blob8_d = nc.dram_tensor("blob8", [PART, max(len8, 1)], f8,
                             kind="ExternalInput")
    blob16_d = nc.dram_tensor("blob16", [PART, max(len16, 1)], bf16,
                              kind="ExternalInput")
    d2_d = nc.dram_tensor("d2", [PART, NTILES_MAX * N_MAX], bf16,
                          kind="ExternalInput")
    sc_d = nc.dram_tensor("scales", [PART, n_slots], fp32, kind="ExternalInput")
    oacc_d = nc.dram_tensor("out_acc", [PART, n_units], fp32,
                            kind="ExternalOutput")

    with TileContext(nc) as tc:
        with (
            tc.tile_pool(name="const", bufs=1) as constp,
            tc.tile_pool(name="blob8p", bufs=3) as blob8p,
            tc.tile_pool(name="blob16p", bufs=3) as blob16p,
            tc.tile_pool(name="epool", bufs=2) as epool,
            tc.tile_pool(name="mpool", bufs=2) as mpool,
            tc.tile_pool(name="accp", bufs=1) as accp,
        ):
            d2_sb = constp.tile([PART, NTILES_MAX * N_MAX], bf16, tag="d2")
            nc.sync.dma_start(out=d2_sb[:], in_=d2_d.ap()[:])
            sc_sb = constp.tile([PART, n_slots], fp32, tag="sc")
            nc.sync.dma_start(out=sc_sb[:], in_=sc_d.ap()[:])
            acc = accp.tile([PART, n_units], fp32, tag="acc")
            nc.vector.memset(acc[:], 0.0)

            for _rep in range(reps):
                u = 0
                for s, sl in enumerate(slots):
                    nt, W, N = sl["ntiles"], sl["W_s"], sl["N_s"]
                    free = sl["free"]
                    if _CAST_DMA:
                        # fp8 in DRAM, upcast to bf16 during the (SWDGE) DMA
                        blob_t = blob8p.tile([PART, max_f8], bf16, tag="b8")
                        nc.gpsimd.dma_start(
                            out=blob_t[:, :free],
                            in_=blob8_d.ap()[:, sl["base"]: sl["base"] + free],
                        )
                    else:
                        if sl["bf16"]:
                            blob_t = blob16p.tile([PART, max_f16], bf16,
                                                  tag="b16")
                            blob_src = blob16_d
                        else:
                            blob_t = blob8p.tile([PART, max_f8], f8, tag="b8")
                            blob_src = blob8_d
                        for rt in range(nt):
                            rows = min(PART, N - rt * PART)
                            nc.sync.dma_start(
                                out=blob_t[0:rows,
                                           rt * W * N: (rt + 1) * W * N],
                                in_=blob_src.ap()[
                                    0:rows,
                                    sl["base"] + rt * W * N:
                                    sl["base"] + (rt + 1) * W * N,
                                ],
                            )
                    e_t = epool.tile([PART, max_e], bf16, tag="e")
                    d2_in = d2_sb[:, : nt * N_MAX].rearrange(
                        "p (r o) -> p r o", r=nt
                    )[:, :, 0:N]
                    e_out = e_t[:, : nt * N].rearrange("p (r o) -> p r o", r=nt)
                    nc.scalar.activation(
                        e_out, d2_in, mybir.ActivationFunctionType.Exp,
                        bias=0.0, scale=sc_sb[:, s: s + 1],
                    )
                    for rt in range(nt):
                        m_t = mpool.tile([PART, max_m], bf16, tag="m")
                        in0 = (
                            e_t[:, rt * N: (rt + 1) * N]
                            .unsqueeze(1)
                            .broadcast_to([PART, W, N])
                        )
                        in1 = blob_t[:, rt * W * N: (rt + 1) * W * N].rearrange(
                            "p (w o) -> p w o", w=W
                        )
                        m_out = m_t[:, : W * N].rearrange(
                            "p (w o) -> p w o", w=W
                        )
                        nc.vector.scalar_tensor_tensor(
                            out=m_out, in0=in0, scalar=1.0, in1=in1,
                            op0=mybir.AluOpType.subtract,
                            op1=mybir.AluOpType.mult,
                            accum_out=acc[:, u: u + 1],
                        )
                        u += 1
            nc.sync.dma_start(out=oacc_d.ap()[:], in_=acc[:])

    nc.compile()
    return nc


def _reduce_outputs(results, slots):
    tot = 0.0
    for res in results:
        acc = np.asarray(res["out_acc"], dtype=np.float64)
        u = 0
        for sl in slots:
            N = sl["N_s"]
            for rt in range(sl["ntiles"]):
                rows = min(PART, N - rt * PART)
                tot += acc[0:rows, u].sum()
                u += 1
    loss = -tot / float(B * N_MAX * T_MAX)
    return np.array(loss, dtype=np.float32)


def kernel(alignments, input_lengths, target_lengths):
    from concourse.bass_utils import run_bass_kernel_spmd

    slots, len8, len16 = _plan(input_lengths, target_lengths)
    in_maps = _host_inputs(alignments, input_lengths, target_lengths, slots,
                           len8, len16)
    nc = _build_bass(slots, reps=1)
    out = run_bass_kernel_spmd(nc, in_maps, core_ids=list(range(N_CORES)))
    return _reduce_outputs(out.results, slots)


if __name__ == "__main__":
    rng = np.random.default_rng(0)
    al = rng.random((B, N_MAX, T_MAX), dtype=np.float32)
    il = rng.integers(N_MAX // 2, N_MAX + 1, size=B).astype(np.int32)
    tl = rng.integers(T_MAX // 2, T_MAX + 1, size=B).astype(np.int32)
    print(kernel(alignments=al, input_lengths=il, target_lengths=tl))
